# revision 6
# baseline (speedup 1.0000x reference)
"""Contrastive (SimCLR-style) loss on 8 Trainium2 NeuronCores.

Math (matches the reference exactly):
  P = concat(projection1, projection2)            # [8192, 256]
  sim = cos_sim(P_i, P_j); diag masked to -1e9; logits = sim / 0.5
  labels = arange(2B)  -> picks the masked diagonal, so
  loss = mean_i( lse_i ) + 2e9, lse_i = log(sum_{j != i} exp(2*sim_ij))

Distribution: data-parallel over the 8192 rows.  Each core normalizes the
full projection matrix on-chip, computes its 1024-row block of the
similarity matrix against all 8192 columns (bf16 matmul, fp32 accum),
streams exp through the scalar engine with fused row-sum accumulation,
subtracts the diagonal term, takes log, and returns per-row lse partial
sums.  The host all-reduces the partials into the scalar loss.
"""

import sys

for _p in ("/opt/trn_rl_repo", "/root/.axon_site/_ro/trn_rl_repo"):
    if _p not in sys.path:
        sys.path.append(_p)

import numpy as np

import concourse.bass as bass
import concourse.bacc as bacc
import concourse.tile as tile
from concourse import mybir
from concourse import bass_utils

F32 = mybir.dt.float32
BF16 = mybir.dt.bfloat16
AF = mybir.ActivationFunctionType
ALU = mybir.AluOpType

N_CORES = 8
B = 8192          # total rows (2 * batch)
D = 256           # projection dim
BLK = B // N_CORES  # 1024 rows per core
M_TILES = BLK // 128      # 8 row tiles per core
R_TILES = B // 128        # 64 row tiles of the full matrix
N_COLS = 512              # matmul free dim (one PSUM bank)
GROUP = 2048              # ACT exp batch (4 PSUM banks)
N_GROUPS = B // GROUP     # 4
N_PER_GROUP = GROUP // N_COLS  # 4
RT_PER_GROUP = R_TILES // N_GROUPS  # 16 row tiles normalized per group




def _emit(tc, p_full, p_blk, lse_out):
    nc = tc.nc
    ctx_pools = []

    persist = tc.alloc_tile_pool(name="persist", bufs=1)
    pin = tc.alloc_tile_pool(name="pin", bufs=4)
    qpool = tc.alloc_tile_pool(name="qpool", bufs=4)
    sqpool = tc.alloc_tile_pool(name="sqpool", bufs=2)
    psum_pool = tc.alloc_tile_pool(name="psum", bufs=2, space="PSUM")
    epool = tc.alloc_tile_pool(name="epool", bufs=2)

    # Persistent on-chip tensors.  QT{0,1} hold the normalized projection
    # matrix transposed ([feature_half, row]) -- the "all-gathered" operand.
    # BT{0,1} hold this core's row block in the same layout (lhsT operand).
    qt0 = persist.tile([128, B], BF16, tag="qt0", name="qt0")
    qt1 = persist.tile([128, B], BF16, tag="qt1", name="qt1")
    bt0 = persist.tile([128, BLK], BF16, tag="bt0", name="bt0")
    bt1 = persist.tile([128, BLK], BF16, tag="bt1", name="bt1")
    stats_b = persist.tile([128, M_TILES], F32, tag="stats_b", name="stats_b")
    rn_b = persist.tile([128, M_TILES], F32, tag="rn_b", name="rn_b")
    selfdot = persist.tile([128, M_TILES], F32, tag="selfdot", name="selfdot")
    stats_f = persist.tile([128, R_TILES], F32, tag="stats_f", name="stats_f")
    rn_f = persist.tile([128, R_TILES], F32, tag="rn_f", name="rn_f")
    sums = persist.tile([128, N_GROUPS * M_TILES], F32, tag="sums", name="sums")
    rowsum = persist.tile([128, M_TILES], F32, tag="rowsum", name="rowsum")
    exps = persist.tile([128, M_TILES], F32, tag="exps", name="exps")
    lse = persist.tile([128, M_TILES], F32, tag="lse", name="lse")

    pf = p_full.rearrange("(t p) d -> t p d", p=128)   # [64, 128, 256]
    pb = p_blk.rearrange("(t p) d -> t p d", p=128)    # [8, 128, 256]

    def normalize_tiles(src, n_tiles, stats, rn, qdst0, qdst1, sdot):
        """DMA fp32 rows, row-normalize to bf16, transpose into qdst tiles."""
        qrows = []
        for t in range(n_tiles):
            # held across the whole strip (reloaded by the scale pass), so the
            # slot ring must cover the strip plus prefetch margin
            p_tile = pin.tile([128, D], F32, name="p_tile", bufs=n_tiles + 2)
            nc.sync.dma_start(out=p_tile, in_=src[t])
            sq = sqpool.tile([128, D], F32, name="sq")
            nc.vector.tensor_mul(sq, p_tile, p_tile)
            nc.vector.tensor_reduce(
                stats[:, t : t + 1], sq, axis=mybir.AxisListType.X, op=ALU.add
            )
            qrows.append(p_tile)
        # rn = 1/sqrt(sumsq) for the whole strip at once
        nrm = sqpool.tile([128, n_tiles], F32, name="nrm")
        nc.scalar.sqrt(nrm, stats[:, 0:n_tiles])
        nc.vector.reciprocal(rn[:, 0:n_tiles], nrm)
        for t in range(n_tiles):
            q = qpool.tile([128, D], BF16, name="q")
            nc.vector.tensor_scalar_mul(q, qrows[t], rn[:, t : t + 1])
            if sdot is not None:
                sqd = sqpool.tile([128, D], F32, name="sqd")
                nc.vector.tensor_mul(sqd, q, q)
                nc.vector.tensor_reduce(
                    sdot[:, t : t + 1], sqd, axis=mybir.AxisListType.X, op=ALU.add
                )
            nc.sync.dma_start_transpose(
                out=qdst0[:, t * 128 : (t + 1) * 128], in_=q[:, 0:128]
            )
            nc.sync.dma_start_transpose(
                out=qdst1[:, t * 128 : (t + 1) * 128], in_=q[:, 128:256]
            )

    # This core's row block (lhsT side + self-dot for the diagonal term).
    normalize_tiles(pb, M_TILES, stats_b, rn_b, bt0, bt1, selfdot)

    # Full matrix, one column-group at a time so the main loop can start
    # before all of it is normalized.
    def normalize_group(g):
        base = g * RT_PER_GROUP
        qrows = []
        for i in range(RT_PER_GROUP):
            t = base + i
            p_tile = pin.tile([128, D], F32, name="pf_tile",
                              bufs=RT_PER_GROUP + 4)
            nc.sync.dma_start(out=p_tile, in_=pf[t])
            sq = sqpool.tile([128, D], F32, name="sqf")
            nc.vector.tensor_mul(sq, p_tile, p_tile)
            nc.vector.tensor_reduce(
                stats_f[:, t : t + 1], sq, axis=mybir.AxisListType.X, op=ALU.add
            )
            qrows.append(p_tile)
        nrm = sqpool.tile([128, RT_PER_GROUP], F32, name="nrmf")
        nc.scalar.sqrt(nrm, stats_f[:, base : base + RT_PER_GROUP])
        nc.vector.reciprocal(rn_f[:, base : base + RT_PER_GROUP], nrm)
        for i in range(RT_PER_GROUP):
            t = base + i
            q = qpool.tile([128, D], BF16, name="qf")
            nc.vector.tensor_scalar_mul(q, qrows[i], rn_f[:, t : t + 1])
            nc.sync.dma_start_transpose(
                out=qt0[:, t * 128 : (t + 1) * 128], in_=q[:, 0:128]
            )
            nc.sync.dma_start_transpose(
                out=qt1[:, t * 128 : (t + 1) * 128], in_=q[:, 128:256]
            )

    normalize_group(0)

    # Main loop: S-block matmuls + fused exp/row-sum.
    for g in range(N_GROUPS):
        if g + 1 < N_GROUPS:
            normalize_group(g + 1)
        for m in range(M_TILES):
            ps = psum_pool.tile([128, GROUP], F32, name="ps")
            for n4 in range(N_PER_GROUP):
                col = g * GROUP + n4 * N_COLS
                for k, (btk, qtk) in enumerate(((bt0, qt0), (bt1, qt1))):
                    nc.tensor.matmul(
                        ps[:, n4 * N_COLS : (n4 + 1) * N_COLS],
                        btk[:, m * 128 : (m + 1) * 128],
                        qtk[:, col : col + N_COLS],
                        start=(k == 0),
                        stop=(k == 1),
                    )
            esc = epool.tile([128, GROUP], BF16, name="esc")
            nc.scalar.activation(
                out=esc,
                in_=ps,
                func=AF.Exp,
                scale=2.0,
                accum_out=sums[:, g * M_TILES + m : g * M_TILES + m + 1],
            )

    # Epilogue: rowsum over groups, drop the diagonal, log, write out.
    sums3 = sums.rearrange("p (g m) -> p m g", g=N_GROUPS)
    nc.vector.tensor_reduce(rowsum, sums3, axis=mybir.AxisListType.X, op=ALU.add)
    nc.scalar.activation(out=exps, in_=selfdot, func=AF.Exp, scale=2.0)
    nc.vector.tensor_tensor(lse, rowsum, exps, op=ALU.subtract)
    nc.scalar.activation(out=lse, in_=lse, func=AF.Ln)
    nc.sync.dma_start(out=lse_out, in_=lse)

    for p in (epool, psum_pool, sqpool, qpool, pin, persist):
        p.release()


_BUILT = None


def _build():
    global _BUILT
    if _BUILT is None:
        nc = bacc.Bacc("TRN2", target_bir_lowering=False, debug=False,
                       num_devices=N_CORES)
        p_full = nc.dram_tensor("p_full", [B, D], F32, kind="ExternalInput").ap()
        p_blk = nc.dram_tensor("p_blk", [BLK, D], F32, kind="ExternalInput").ap()
        lse_out = nc.dram_tensor("lse_out", [128, M_TILES], F32,
                                 kind="ExternalOutput").ap()
        with tile.TileContext(nc) as tc:
            _emit(tc, p_full, p_blk, lse_out)
        nc.finalize()
        _BUILT = nc
    return _BUILT


def run_on_hw(P, **spmd_kwargs):
    nc = _build()
    in_maps = [
        {"p_full": P, "p_blk": P[c * BLK : (c + 1) * BLK]} for c in range(N_CORES)
    ]
    return bass_utils.run_bass_kernel_spmd(
        nc, in_maps, core_ids=list(range(N_CORES)), **spmd_kwargs
    )


def kernel(embedding1, embedding2, projection1, projection2):
    # embeddings are unused by the reference computation
    P = np.ascontiguousarray(
        np.concatenate([projection1, projection2], axis=0), dtype=np.float32
    )
    res = run_on_hw(P)
    total = 0.0
    for c in range(N_CORES):
        total += res.results[c]["lse_out"].astype(np.float64).sum()
    return np.float32(2.0e9 + total / B)


# revision 8
# speedup vs baseline: 1.9635x; 1.9635x over previous
"""Contrastive (SimCLR-style) loss on 8 Trainium2 NeuronCores.

Math (matches the reference exactly):
  P = concat(projection1, projection2)            # [8192, 256]
  sim = cos_sim(P_i, P_j); diag masked to -1e9; logits = sim / 0.5
  labels = arange(2B)  -> picks the masked diagonal, so
  loss = -mean_i( logp_ii ),  logp_ii = f32(-2e9 - lse_i),
  lse_i = log(sum_{j != i} exp(2*sim_ij))

Distribution: data-parallel over the 8192 rows.  Each core receives the
full projection matrix (row-major fp32 for norms + pre-transposed bf16
for the matmul operand) plus its own 1024-row block.  On chip it:
  - computes row norms (DVE square+reduce, Newton rsqrt -- no ScalarE),
  - scales the transposed operand by 1/norm (bf16),
  - matmuls its row block against all 8192 columns (bf16, fp32 PSUM),
  - streams exp through ScalarE with fused row-sum accumulation,
  - subtracts the diagonal term and takes log.
Host all-reduces the per-row lse partials and applies the reference's
fp32 arithmetic for the final mean.
"""

import sys

for _p in ("/opt/trn_rl_repo", "/root/.axon_site/_ro/trn_rl_repo"):
    if _p not in sys.path:
        sys.path.append(_p)

import numpy as np

import concourse.bacc as bacc
import concourse.tile as tile
from concourse import mybir
from concourse import bass_utils

F32 = mybir.dt.float32
BF16 = mybir.dt.bfloat16
I32 = mybir.dt.int32
AF = mybir.ActivationFunctionType
ALU = mybir.AluOpType

N_CORES = 8
B = 8192          # total rows (2 * batch)
D = 256           # projection dim
BLK = B // N_CORES        # 1024 rows per core
M_TILES = BLK // 128      # 8 row tiles per core
N_COLS = 512              # matmul free dim (one PSUM bank)
GROUP = 2048              # ACT exp batch (4 PSUM banks) = one column group
N_GROUPS = B // GROUP     # 4
N_PER_GROUP = GROUP // N_COLS  # 4
U = 16                    # consecutive rows per partition in stats loads
RSQRT_MAGIC = 0x5F3759DF


def _newton_rsqrt(nc, pool, out_rn, s):
    """out_rn = 1/sqrt(s), entirely on VectorE (fp32).

    Quake-style bit seed + 2 Newton iterations (~5e-6 rel err).  Keeps
    ScalarE free for exp and avoids sqrt<->exp table reloads.
    """
    p, w = s.shape
    ibits = pool.tile([p, w], I32, name="ibits", tag="rsq_i", bufs=2)
    nc.vector.tensor_scalar(
        out=ibits, in0=s.bitcast(I32), scalar1=1, scalar2=None,
        op0=ALU.arith_shift_right,
    )
    nc.vector.tensor_scalar(
        out=ibits, in0=ibits, scalar1=-1, scalar2=RSQRT_MAGIC,
        op0=ALU.mult, op1=ALU.add,
    )
    y = ibits.bitcast(F32)
    t1 = pool.tile([p, w], F32, name="t1", tag="rsq_t1", bufs=2)
    for _ in range(2):
        nc.vector.tensor_mul(t1, y, y)
        nc.vector.tensor_mul(t1, t1, s)
        nc.vector.tensor_scalar(
            out=t1, in0=t1, scalar1=-0.5, scalar2=1.5,
            op0=ALU.mult, op1=ALU.add,
        )
        nc.vector.tensor_mul(y, y, t1)
    nc.vector.tensor_copy(out_rn, y)


def _emit(tc, p_stats, pt, p_blk, lse_out):
    nc = tc.nc

    persist = tc.alloc_tile_pool(name="persist", bufs=1)
    pin = tc.alloc_tile_pool(name="pin", bufs=2)
    work = tc.alloc_tile_pool(name="work", bufs=2)
    dram = tc.alloc_tile_pool(name="dram", bufs=1, space="DRAM")
    psum_pool = tc.alloc_tile_pool(name="psum", bufs=2, space="PSUM")
    epool = tc.alloc_tile_pool(name="epool", bufs=2)

    # Persistent tensors
    qt0 = persist.tile([128, B], BF16, tag="qt0", name="qt0")
    qt1 = persist.tile([128, B], BF16, tag="qt1", name="qt1")
    bt0 = persist.tile([128, BLK], BF16, tag="bt0", name="bt0")
    bt1 = persist.tile([128, BLK], BF16, tag="bt1", name="bt1")
    q_b = persist.tile([128, M_TILES, D], BF16, tag="q_b", name="q_b")
    rn_f = persist.tile([128, 64], F32, tag="rn_f", name="rn_f")
    rn_fb = persist.tile([128, 64], BF16, tag="rn_fb", name="rn_fb")
    rn_b = persist.tile([128, M_TILES], F32, tag="rn_b", name="rn_b")
    selfdot = persist.tile([128, M_TILES], F32, tag="selfdot", name="selfdot")
    sums = persist.tile([128, N_GROUPS * M_TILES], F32, tag="sums", name="sums")
    rowsum = persist.tile([128, M_TILES], F32, tag="rowsum", name="rowsum")
    exps = persist.tile([128, M_TILES], F32, tag="exps", name="exps")
    lse = persist.tile([128, M_TILES], F32, tag="lse", name="lse")
    dram_rn = dram.tile([B], BF16, tag="dram_rn", name="dram_rn")

    # ---- This core's row block: norms, scale, self-dot, transpose ----
    pb = p_blk.rearrange("(t p) d -> t p d", p=128)    # [8, 128, 256]
    blk = persist.tile([128, M_TILES, D], F32, tag="blk", name="blk")
    for t in range(M_TILES):
        nc.sync.dma_start(out=blk[:, t, :], in_=pb[t])
    sq_b = work.tile([128, M_TILES, D], BF16, name="sq_b", tag="sq_b", bufs=1)
    nc.vector.tensor_mul(sq_b, blk, blk)
    stats_b = work.tile([128, M_TILES], F32, name="stats_b", tag="stats_b", bufs=1)
    nc.vector.tensor_reduce(stats_b, sq_b, axis=mybir.AxisListType.X, op=ALU.add)
    _newton_rsqrt(nc, work, rn_b, stats_b)
    for t in range(M_TILES):
        nc.vector.tensor_scalar_mul(q_b[:, t, :], blk[:, t, :], rn_b[:, t : t + 1])
    sq_b2 = work.tile([128, M_TILES, D], BF16, name="sq_b2", tag="sq_b", bufs=1)
    nc.vector.tensor_mul(sq_b2, q_b, q_b)
    nc.vector.tensor_reduce(selfdot, sq_b2, axis=mybir.AxisListType.X, op=ALU.add)
    for t in range(M_TILES):
        nc.sync.dma_start_transpose(
            out=bt0[:, t * 128 : (t + 1) * 128], in_=q_b[:, t, 0:128]
        )
        nc.sync.dma_start_transpose(
            out=bt1[:, t * 128 : (t + 1) * 128], in_=q_b[:, t, 128:256]
        )

    # ---- Full-matrix norms + scaled transposed operand, one group at a
    # time (group g covers columns [2048g, 2048(g+1)) = rows with the
    # same indices; the u=16 interleave keeps j-order identity) ----
    # stats load: row j = 2048t + 16p + u  ->  tile t, partition p, slot u
    ps4 = p_stats.rearrange("(t p u) d -> t p (u d)", p=128, u=U)  # [4,128,4096]
    # rn store: dram_rn[2048t + 16p + u] <- rn_small[p, 16t + u]
    rn_store = dram_rn.rearrange("(t p u) -> t p u", p=128, u=U)   # [4,128,16]

    def normalize_group(g):
        pst = pin.tile([128, U * D], F32, name="pst", tag="pst", bufs=2)
        nc.sync.dma_start(out=pst, in_=ps4[g])
        sq = work.tile([128, U * D], BF16, name="sq", tag="sq", bufs=2)
        nc.vector.tensor_mul(sq, pst, pst)
        nc.vector.tensor_reduce(
            rn_f[:, g * U : (g + 1) * U],
            sq.rearrange("p (u d) -> p u d", u=U),
            axis=mybir.AxisListType.X,
            op=ALU.add,
        )
        _newton_rsqrt(
            nc, work, rn_f[:, g * U : (g + 1) * U], rn_f[:, g * U : (g + 1) * U]
        )
        nc.vector.tensor_copy(
            rn_fb[:, g * U : (g + 1) * U], rn_f[:, g * U : (g + 1) * U]
        )
        nc.sync.dma_start(
            out=rn_store[g],
            in_=rn_fb[:, g * U : (g + 1) * U],
        )
        rnb = work.tile([128, GROUP], BF16, name="rnb", tag="rnb", bufs=2)
        nc.sync.dma_start(
            out=rnb,
            in_=dram_rn[g * GROUP : (g + 1) * GROUP].partition_broadcast(128),
        )
        for k, qtk in enumerate((qt0, qt1)):
            ptc = pin.tile([128, GROUP], BF16, name="ptc", tag="ptc", bufs=3)
            nc.sync.dma_start(
                out=ptc,
                in_=pt[k * 128 : (k + 1) * 128, g * GROUP : (g + 1) * GROUP],
            )
            nc.vector.tensor_mul(
                qtk[:, g * GROUP : (g + 1) * GROUP], ptc, rnb
            )

    normalize_group(0)

    # ---- Main loop: S-block matmuls + fused exp/row-sum ----
    for g in range(N_GROUPS):
        if g + 1 < N_GROUPS:
            normalize_group(g + 1)
        for m in range(M_TILES):
            ps = psum_pool.tile([128, GROUP], F32, name="ps")
            for n4 in range(N_PER_GROUP):
                col = g * GROUP + n4 * N_COLS
                for k, (btk, qtk) in enumerate(((bt0, qt0), (bt1, qt1))):
                    nc.tensor.matmul(
                        ps[:, n4 * N_COLS : (n4 + 1) * N_COLS],
                        btk[:, m * 128 : (m + 1) * 128],
                        qtk[:, col : col + N_COLS],
                        start=(k == 0),
                        stop=(k == 1),
                    )
            esc = epool.tile([128, GROUP], BF16, name="esc")
            nc.scalar.activation(
                out=esc,
                in_=ps,
                func=AF.Exp,
                scale=2.0,
                accum_out=sums[:, g * M_TILES + m : g * M_TILES + m + 1],
            )

    # ---- Epilogue: rowsum over groups, drop diagonal, log, write out ----
    sums3 = sums.rearrange("p (g m) -> p m g", g=N_GROUPS)
    nc.vector.tensor_reduce(rowsum, sums3, axis=mybir.AxisListType.X, op=ALU.add)
    nc.scalar.activation(out=exps, in_=selfdot, func=AF.Exp, scale=2.0)
    nc.vector.tensor_tensor(lse, rowsum, exps, op=ALU.subtract)
    nc.scalar.activation(out=lse, in_=lse, func=AF.Ln)
    nc.sync.dma_start(out=lse_out, in_=lse)

    for p in (epool, psum_pool, dram, work, pin, persist):
        p.release()


_BUILT = None


def _build():
    global _BUILT
    if _BUILT is None:
        nc = bacc.Bacc("TRN2", target_bir_lowering=False, debug=False,
                       num_devices=N_CORES)
        p_stats = nc.dram_tensor("p_stats", [B, D], F32, kind="ExternalInput").ap()
        pt = nc.dram_tensor("pt", [D, B], BF16, kind="ExternalInput").ap()
        p_blk = nc.dram_tensor("p_blk", [BLK, D], F32, kind="ExternalInput").ap()
        lse_out = nc.dram_tensor("lse_out", [128, M_TILES], F32,
                                 kind="ExternalOutput").ap()
        with tile.TileContext(nc) as tc:
            _emit(tc, p_stats, pt, p_blk, lse_out)
        nc.finalize()
        _BUILT = nc
    return _BUILT


def run_on_hw(P, **spmd_kwargs):
    import jax.numpy as jnp

    nc = _build()
    pt_bf16 = np.asarray(jnp.asarray(np.ascontiguousarray(P.T)).astype(jnp.bfloat16))
    in_maps = [
        {
            "p_stats": P,
            "pt": pt_bf16,
            "p_blk": np.ascontiguousarray(P[c * BLK : (c + 1) * BLK]),
        }
        for c in range(N_CORES)
    ]
    return bass_utils.run_bass_kernel_spmd(
        nc, in_maps, core_ids=list(range(N_CORES)), **spmd_kwargs
    )


def kernel(embedding1, embedding2, projection1, projection2):
    import jax.numpy as jnp

    # embeddings are unused by the reference computation
    P = np.ascontiguousarray(
        np.concatenate([projection1, projection2], axis=0), dtype=np.float32
    )
    res = run_on_hw(P)
    # reassemble per-row lse: core c, tile column m, partition p ->
    # global row c*1024 + m*128 + p
    lse_rows = np.empty(B, np.float32)
    for c in range(N_CORES):
        arr = np.asarray(res.results[c]["lse_out"])  # [128, M_TILES]
        lse_rows[c * BLK : (c + 1) * BLK] = arr.T.reshape(-1)
    # Reference fp32 semantics: logp_ii = f32(-2e9 - lse_i) (== -2e9 for
    # any |lse| < 128), then loss = -mean(logp) with the platform's XLA
    # fp32 reduction -- reproduce it bit-for-bit.
    logp = (np.float32(-2.0e9) - lse_rows).astype(np.float32)
    loss = -jnp.mean(jnp.asarray(logp))
    return np.asarray(loss)


# revision 10
# speedup vs baseline: 2.0837x; 1.0612x over previous
"""Contrastive (SimCLR-style) loss on 8 Trainium2 NeuronCores.

Math (matches the reference exactly):
  P = concat(projection1, projection2)            # [8192, 256]
  sim = cos_sim(P_i, P_j); diag masked to -1e9; logits = sim / 0.5
  labels = arange(2B)  -> picks the masked diagonal, so
  loss = -mean_i( logp_ii ),  logp_ii = f32(-2e9 - lse_i),
  lse_i = log(sum_{j != i} exp(2*sim_ij))

Distribution: data-parallel over the 8192 rows.  Each core receives the
full projection matrix (row-major fp32 for norms + pre-transposed bf16
for the matmul operand) plus its own 1024-row block.  On chip it:
  - computes row norms (DVE square+reduce, Newton rsqrt -- no ScalarE),
  - scales the transposed operand by 1/norm (bf16),
  - matmuls its row block against all 8192 columns (bf16, fp32 PSUM),
  - streams exp through ScalarE with fused row-sum accumulation,
  - subtracts the diagonal term and takes log.
Host all-reduces the per-row lse partials and applies the reference's
fp32 arithmetic for the final mean.
"""

import sys

for _p in ("/opt/trn_rl_repo", "/root/.axon_site/_ro/trn_rl_repo"):
    if _p not in sys.path:
        sys.path.append(_p)

import numpy as np

import concourse.bacc as bacc
import concourse.tile as tile
from concourse import mybir
from concourse import bass_utils

F32 = mybir.dt.float32
BF16 = mybir.dt.bfloat16
I32 = mybir.dt.int32
AF = mybir.ActivationFunctionType
ALU = mybir.AluOpType

N_CORES = 8
B = 8192          # total rows (2 * batch)
D = 256           # projection dim
BLK = B // N_CORES        # 1024 rows per core
M_TILES = BLK // 128      # 8 row tiles per core
N_COLS = 512              # matmul free dim (one PSUM bank)
GROUP = 2048              # ACT exp batch (4 PSUM banks) = one column group
N_GROUPS = B // GROUP     # 4
N_PER_GROUP = GROUP // N_COLS  # 4
U = 16                    # consecutive rows per partition in stats loads
RSQRT_MAGIC = 0x5F3759DF


def _newton_rsqrt(nc, pool, out_rn, s):
    """out_rn = 1/sqrt(s), entirely on VectorE (fp32).

    Quake-style bit seed + 2 Newton iterations (~5e-6 rel err).  Keeps
    ScalarE free for exp and avoids sqrt<->exp table reloads.
    """
    p, w = s.shape
    ibits = pool.tile([p, w], I32, name="ibits", tag="rsq_i", bufs=2)
    nc.vector.tensor_scalar(
        out=ibits, in0=s.bitcast(I32), scalar1=1, scalar2=None,
        op0=ALU.arith_shift_right,
    )
    nc.vector.tensor_scalar(
        out=ibits, in0=ibits, scalar1=-1, scalar2=RSQRT_MAGIC,
        op0=ALU.mult, op1=ALU.add,
    )
    y = ibits.bitcast(F32)
    t1 = pool.tile([p, w], F32, name="t1", tag="rsq_t1", bufs=2)
    for _ in range(2):
        nc.vector.tensor_mul(t1, y, y)
        nc.vector.tensor_mul(t1, t1, s)
        nc.vector.tensor_scalar(
            out=t1, in0=t1, scalar1=-0.5, scalar2=1.5,
            op0=ALU.mult, op1=ALU.add,
        )
        nc.vector.tensor_mul(y, y, t1)
    nc.vector.tensor_copy(out_rn, y)


def _emit(tc, p_stats, pt, p_blk, eye_in, lse_out):
    nc = tc.nc

    persist = tc.alloc_tile_pool(name="persist", bufs=1)
    pin = tc.alloc_tile_pool(name="pin", bufs=2)
    work = tc.alloc_tile_pool(name="work", bufs=2)
    dram = tc.alloc_tile_pool(name="dram", bufs=1, space="DRAM")
    epool = tc.alloc_tile_pool(name="epool", bufs=2)

    # Persistent tensors
    qt0 = persist.tile([128, B], BF16, tag="qt0", name="qt0")
    qt1 = persist.tile([128, B], BF16, tag="qt1", name="qt1")
    bt0 = persist.tile([128, BLK], BF16, tag="bt0", name="bt0")
    bt1 = persist.tile([128, BLK], BF16, tag="bt1", name="bt1")
    q_b = persist.tile([128, M_TILES, D], BF16, tag="q_b", name="q_b")
    rn_f = persist.tile([128, 64], F32, tag="rn_f", name="rn_f")
    rn_b = persist.tile([128, M_TILES], F32, tag="rn_b", name="rn_b")
    selfdot = persist.tile([128, M_TILES], F32, tag="selfdot", name="selfdot")
    sums = persist.tile([128, N_GROUPS * M_TILES], F32, tag="sums", name="sums")
    rowsum = persist.tile([128, M_TILES], F32, tag="rowsum", name="rowsum")
    exps = persist.tile([128, M_TILES], F32, tag="exps", name="exps")
    lse = persist.tile([128, M_TILES], F32, tag="lse", name="lse")
    dram_rn = dram.tile([B], F32, tag="dram_rn", name="dram_rn")

    # ---- This core's row block: norms, scale, self-dot, transpose ----
    pb = p_blk.rearrange("(t p) d -> t p d", p=128)    # [8, 128, 256]
    blk = persist.tile([128, M_TILES, D], F32, tag="blk", name="blk")
    eye = persist.tile([128, 128], BF16, tag="eye", name="eye")
    nc.gpsimd.dma_start(out=eye, in_=eye_in)
    for t in range(M_TILES):
        nc.gpsimd.dma_start(out=blk[:, t, :], in_=pb[t])
    sq_b = work.tile([128, M_TILES, D], BF16, name="sq_b", tag="sq_b", bufs=1)
    nc.vector.tensor_mul(sq_b, blk, blk)
    stats_b = work.tile([128, M_TILES], F32, name="stats_b", tag="stats_b", bufs=1)
    nc.vector.tensor_reduce(stats_b, sq_b, axis=mybir.AxisListType.X, op=ALU.add)
    _newton_rsqrt(nc, work, rn_b, stats_b)
    for t in range(M_TILES):
        nc.vector.tensor_scalar_mul(q_b[:, t, :], blk[:, t, :], rn_b[:, t : t + 1])
    sq_b2 = work.tile([128, M_TILES, D], BF16, name="sq_b2", tag="sq_b", bufs=1)
    nc.vector.tensor_mul(sq_b2, q_b, q_b)
    nc.vector.tensor_reduce(selfdot, sq_b2, axis=mybir.AxisListType.X, op=ALU.add)
    # Transpose the block on the (otherwise idle) tensor engine; copy the
    # PSUM results to SBUF on the scalar engine.  This keeps the slow DMA
    # xbar out of the picture and frees the main loop from DMA-queue deps.
    tp_psum = tc.alloc_tile_pool(name="tp_psum", bufs=4, space="PSUM")
    for t in range(M_TILES):
        for half, btk in ((0, bt0), (1, bt1)):
            tp = tp_psum.tile([128, 128], BF16, name="tp")
            nc.tensor.transpose(tp, q_b[:, t, half * 128 : half * 128 + 128], eye)
            nc.scalar.copy(out=btk[:, t * 128 : (t + 1) * 128], in_=tp)
    tp_psum.release()
    psum_pool = tc.alloc_tile_pool(name="psum", bufs=2, space="PSUM")

    # ---- Full-matrix norms + scaled transposed operand, one group at a
    # time (group g covers columns [2048g, 2048(g+1)) = rows with the
    # same indices; the u=16 interleave keeps j-order identity) ----
    # stats load: row j = 2048t + 16p + u  ->  tile t, partition p, slot u
    ps4 = p_stats.rearrange("(t p u) d -> t p (u d)", p=128, u=U)  # [4,128,4096]
    # rn store: dram_rn[2048t + 16p + u] <- rn_small[p, 16t + u]
    rn_store = dram_rn.rearrange("(t p u) -> t p u", p=128, u=U)   # [4,128,16]

    def normalize_group(g):
        pst = pin.tile([128, U * D], F32, name="pst", tag="pst", bufs=2)
        nc.sync.dma_start(out=pst, in_=ps4[g])
        sq = work.tile([128, U * D], BF16, name="sq", tag="sq", bufs=2)
        nc.vector.tensor_mul(sq, pst, pst)
        nc.vector.tensor_reduce(
            rn_f[:, g * U : (g + 1) * U],
            sq.rearrange("p (u d) -> p u d", u=U),
            axis=mybir.AxisListType.X,
            op=ALU.add,
        )
        _newton_rsqrt(
            nc, work, rn_f[:, g * U : (g + 1) * U], rn_f[:, g * U : (g + 1) * U]
        )
        nc.sync.dma_start(
            out=rn_store[g],
            in_=rn_f[:, g * U : (g + 1) * U].rearrange("p (t u) -> p t u", u=U),
        )
        rnb = work.tile([128, GROUP], F32, name="rnb", tag="rnb", bufs=2)
        nc.sync.dma_start(
            out=rnb,
            in_=dram_rn[g * GROUP : (g + 1) * GROUP].partition_broadcast(128),
        )
        for k, qtk in enumerate((qt0, qt1)):
            ptc = pin.tile([128, GROUP], F32, name="ptc", tag="ptc", bufs=4)
            nc.gpsimd.dma_start(
                out=ptc,
                in_=pt[k * 128 : (k + 1) * 128, g * GROUP : (g + 1) * GROUP],
            )
            nc.vector.tensor_mul(
                qtk[:, g * GROUP : (g + 1) * GROUP], ptc, rnb
            )

    normalize_group(0)

    # ---- Main loop: S-block matmuls + fused exp/row-sum ----
    for g in range(N_GROUPS):
        if g + 1 < N_GROUPS:
            normalize_group(g + 1)
        for m in range(M_TILES):
            ps = psum_pool.tile([128, GROUP], F32, name="ps")
            for n4 in range(N_PER_GROUP):
                col = g * GROUP + n4 * N_COLS
                for k, (btk, qtk) in enumerate(((bt0, qt0), (bt1, qt1))):
                    nc.tensor.matmul(
                        ps[:, n4 * N_COLS : (n4 + 1) * N_COLS],
                        btk[:, m * 128 : (m + 1) * 128],
                        qtk[:, col : col + N_COLS],
                        start=(k == 0),
                        stop=(k == 1),
                    )
            esc = epool.tile([128, GROUP], BF16, name="esc")
            nc.scalar.activation(
                out=esc,
                in_=ps,
                func=AF.Exp,
                scale=2.0,
                accum_out=sums[:, g * M_TILES + m : g * M_TILES + m + 1],
            )

    # ---- Epilogue: rowsum over groups, drop diagonal, log, write out ----
    sums3 = sums.rearrange("p (g m) -> p m g", g=N_GROUPS)
    nc.vector.tensor_reduce(rowsum, sums3, axis=mybir.AxisListType.X, op=ALU.add)
    nc.scalar.activation(out=exps, in_=selfdot, func=AF.Exp, scale=2.0)
    nc.vector.tensor_tensor(lse, rowsum, exps, op=ALU.subtract)
    nc.scalar.activation(out=lse, in_=lse, func=AF.Ln)
    nc.sync.dma_start(out=lse_out, in_=lse)

    for p in (epool, psum_pool, dram, work, pin, persist):
        p.release()


_BUILT = None


def _build():
    global _BUILT
    if _BUILT is None:
        nc = bacc.Bacc("TRN2", target_bir_lowering=False, debug=False,
                       num_devices=N_CORES)
        p_stats = nc.dram_tensor("p_stats", [B, D], F32, kind="ExternalInput").ap()
        pt = nc.dram_tensor("pt", [D, B], F32, kind="ExternalInput").ap()
        eye = nc.dram_tensor("eye", [128, 128], BF16, kind="ExternalInput").ap()
        p_blk = nc.dram_tensor("p_blk", [BLK, D], F32, kind="ExternalInput").ap()
        lse_out = nc.dram_tensor("lse_out", [128, M_TILES], F32,
                                 kind="ExternalOutput").ap()
        with tile.TileContext(nc) as tc:
            _emit(tc, p_stats, pt, p_blk, eye, lse_out)
        nc.finalize()
        _BUILT = nc
    return _BUILT


def run_on_hw(P, **spmd_kwargs):
    import jax.numpy as jnp

    nc = _build()
    pt_f32 = np.ascontiguousarray(P.T)
    eye = np.asarray(jnp.eye(128, dtype=jnp.bfloat16))
    in_maps = [
        {
            "p_stats": P,
            "pt": pt_f32,
            "p_blk": np.ascontiguousarray(P[c * BLK : (c + 1) * BLK]),
            "eye": eye,
        }
        for c in range(N_CORES)
    ]
    return bass_utils.run_bass_kernel_spmd(
        nc, in_maps, core_ids=list(range(N_CORES)), **spmd_kwargs
    )


def kernel(embedding1, embedding2, projection1, projection2):
    import jax.numpy as jnp

    # embeddings are unused by the reference computation
    P = np.ascontiguousarray(
        np.concatenate([projection1, projection2], axis=0), dtype=np.float32
    )
    res = run_on_hw(P)
    # reassemble per-row lse: core c, tile column m, partition p ->
    # global row c*1024 + m*128 + p
    lse_rows = np.empty(B, np.float32)
    for c in range(N_CORES):
        arr = np.asarray(res.results[c]["lse_out"])  # [128, M_TILES]
        lse_rows[c * BLK : (c + 1) * BLK] = arr.T.reshape(-1)
    # Reference fp32 semantics: logp_ii = f32(-2e9 - lse_i) (== -2e9 for
    # any |lse| < 128), then loss = -mean(logp) with the platform's XLA
    # fp32 reduction -- reproduce it bit-for-bit.
    logp = (np.float32(-2.0e9) - lse_rows).astype(np.float32)
    loss = -jnp.mean(jnp.asarray(logp))
    return np.asarray(loss)
